# revision 35
# baseline (speedup 1.0000x reference)
"""Trainium2 Bass kernel for BiDACPI (GAT + CNN + bidirectional attention).

Data-parallel over batch b=16 across 8 NeuronCores (2 graphs per core).
Self-contained: hardcodes all shapes; host-side preprocessing only reshapes /
transposes weights and converts index tensors.

v3: attention logits z = mask + src_i + dst_j are built ON THE PE
(fp8-DoubleRow identity x mask matmul + K=2 rank-2 matmul of
[dst;ones]^T [ones;src]), eliminating the DVE broadcast/accumulator ops
that dominated v2. Heads are processed in pairs so softmax-normalize +
elu run on packed [128, 512] tiles. fp16 matmuls; f32 on DVE
scalar_tensor_tensor paths (fp16 is slower there); fp16 only where DVE
tensor_scalar/copy 4x modes apply. Pooling is folded into activation
accum_out (masks are spec'd fill=ones).
"""
import numpy as np

import concourse.bass as bass
import concourse.mybir as mybir
import concourse.tile as tile
from concourse import bacc

F32 = mybir.dt.float32
F16 = mybir.dt.float16
F8 = mybir.dt.float8e5
F8E4 = mybir.dt.float8e4
I32 = mybir.dt.int32
AT = mybir.AluOpType
AF = mybir.ActivationFunctionType
DR = mybir.MatmulPerfMode.DoubleRow

# Problem constants
B = 16
NCORES = 8
G = B // NCORES          # graphs per core
N = 512                  # atoms per graph
L = 1024                 # amino length
CD = 128                 # comp_dim
PD = 128                 # prot_dim
GD = 64                  # gat_dim
H = 4                    # heads
LAT = 128                # latent
NA = 100                 # num_atom
NAM = 30                 # num_amino
LC = 3                   # conv layers
KW = 11                  # conv kernel width
ALPHA = 0.2
MASKNEG = -28672.0       # fp8e5-exact additive mask
NT = N // 128            # 4 j-chunks
PADL = KW // 2
PVW = PADL + L + PADL + 2  # padded pv width (1036, even)

MASK_FP8_DR = True       # fp8e5 identity-mask matmul
LEAKY_SPLIT = 5          # unused
CONV_FP8 = False          # conv band matmuls in fp8e4m3 + DoubleRow i-pairs
NPR = (KW + 1) // 2      # 6 DoubleRow pairs
PV8W = 1040              # fp8 dual-plane width (16B-aligned plane stride)


def build_core_program(debug=False, mm_bf16=False, dump=False):
    """debug=True builds the CoreSim-compatible variant (no Prelu; no
    activation accum_out)."""
    if debug:
        nc = bacc.Bacc(None, target_bir_lowering=False, debug=True)
    else:
        nc = bacc.Bacc(None)
    MD = F16
    use_prelu = not debug

    # ---- DRAM I/O ----
    d_atoms = nc.dram_tensor("atoms_f", [G, N], MD, kind="ExternalInput")
    d_amask = nc.dram_tensor("atoms_mask", [G, N], F32, kind="ExternalInput")
    # ladj8[g, p, t, i] = additive mask for edge j=t*128+p -> i (fp8e5)
    d_ladj8 = nc.dram_tensor("ladj8", [G, 128, NT, N], F8,
                             kind="ExternalInput")
    d_amino = nc.dram_tensor("amino_f", [G, L], MD, kind="ExternalInput")
    d_pmask = nc.dram_tensor("amino_mask", [G, L], F32, kind="ExternalInput")
    # packed weight blobs (one DMA each): offsets must match preprocess()
    W16O = dict(Eat=0, Eam=128, Wg=256, Wa2_98=512, Wa1_98=610, Wgo=708,
                Wgoa4=964, I16=972, E2=1100, Wc=1228, Wa=1356,
                MiT=1484)
    W16N = 1484 + LC * KW * PD
    WFO = dict(b98w=0, b98r=1, bc=2, ba=3, pw=4, pb=6)
    WFN = 7
    d_blob16 = nc.dram_tensor("blob16", [128, W16N], MD,
                              kind="ExternalInput")
    d_blobf = nc.dram_tensor("blobf", [128, WFN], F32, kind="ExternalInput")
    d_blob8 = nc.dram_tensor("blob8", [128, 128 + LC * NPR * 2 * PD], F8E4,
                             kind="ExternalInput")
    d_cb = nc.dram_tensor("conv_b", [LC, 1], F32, kind="ExternalInput")
    d_out = nc.dram_tensor("out", [G, 1], F32, kind="ExternalOutput")
    d_dbg = {}
    if dump:
        for nm, shp in [("U0", [128, NT, N]), ("zm0", [128, NT, N]),
                        ("m01", [128, N]), ("m23", [128, N]),
                        ("xT", [128, N]), ("rinv0", [1, N]),
                        ("pv3", [128, PVW]), ("comp", [LAT, 1]),
                        ("prot", [LAT, 1])]:
            d_dbg[nm] = nc.dram_tensor("dbg_" + nm, shp, F32,
                                       kind="ExternalOutput")

    with tile.TileContext(nc) as tc:
        with (
            tc.tile_pool(name="const", bufs=1) as cpool,
            tc.tile_pool(name="work", bufs=1) as wpool,
            tc.tile_pool(name="att", bufs=1) as apool,
            tc.tile_pool(name="rows", bufs=1) as rpool,
            tc.tile_pool(name="ps_sq", bufs=1, space="PSUM") as pssq,
            tc.tile_pool(name="ps_hp", bufs=2, space="PSUM") as pshp,
            tc.tile_pool(name="ps_z", bufs=2, space="PSUM") as psz,
            tc.tile_pool(name="ps_cv", bufs=1, space="PSUM") as pscv,
        ):
            # queue heads: iota first on gpsimd (gates one-hots), const
            # memsets first on DVE
            ioi = cpool.tile([128, L], I32)
            nc.gpsimd.iota(ioi, pattern=[[0, L]], base=0,
                           channel_multiplier=1)
            ones_col = cpool.tile([128, 1], F32)
            nc.vector.memset(ones_col, 1.0)
            ones16c = cpool.tile([128, 2], MD)
            nc.vector.memset(ones16c, 1.0)
            onesrow = cpool.tile([1, 128], MD)
            nc.vector.memset(onesrow, 1.0)
            iof = cpool.tile([128, L], F32)
            nc.vector.tensor_copy(iof, ioi)

            # ---- input DMAs (phase-1-critical first on each queue) ----
            g_in = []
            rows_in = []
            for g in range(G):
                arow = rpool.tile([1, N], MD, tag="gin1k", bufs=4,
                                  name="arow")
                nc.sync.dma_start(out=arow, in_=d_atoms[g : g + 1, :])
                prow = rpool.tile([1, L], MD, tag="gin2k", bufs=4,
                                  name="prow")
                nc.sync.dma_start(out=prow, in_=d_amino[g : g + 1, :])
                rows_in.append((arow, prow))

            # ---- weights: 3 blob DMAs + cb ----
            blob16 = cpool.tile([128, W16N], MD, tag="blob16", name="blob16")
            nc.sync.dma_start(out=blob16, in_=d_blob16[:, :])
            blobf = cpool.tile([128, WFN], F32, tag="blobf", name="blobf")
            nc.sync.dma_start(out=blobf, in_=d_blobf[:, :])
            blob8 = cpool.tile([128, 128 + LC * NPR * 2 * PD], F8E4,
                               tag="blob8", name="blob8")
            nc.scalar.dma_start(out=blob8, in_=d_blob8[:, :])
            cb = cpool.tile([128, LC], F32)
            nc.gpsimd.dma_start(
                out=cb,
                in_=bass.AP(tensor=d_cb, offset=0,
                            ap=[[0, 128], [1, LC], [0, 1]]),
            )
            for g in range(G):
                ladj8 = apool.tile([128, NT, N], F8, tag=f"ladj{g}",
                                   name="ladj8")
                if g == 0:
                    nc.gpsimd.dma_start(out=ladj8, in_=d_ladj8[g])
                else:
                    nc.scalar.dma_start(out=ladj8, in_=d_ladj8[g])
                amrow = rpool.tile([1, N], F32, tag="gin2kf", bufs=4,
                                   name="amrow")
                nc.gpsimd.dma_start(out=amrow, in_=d_amask[g : g + 1, :])
                pmrow = rpool.tile([1, L], F32, tag="gin4kf", bufs=4,
                                   name="pmrow")
                nc.gpsimd.dma_start(out=pmrow, in_=d_pmask[g : g + 1, :])
                g_in.append((rows_in[g][0], rows_in[g][1], ladj8, amrow,
                             pmrow))

            def w16(nm, n, rows=128):
                return blob16[0:rows, W16O[nm] : W16O[nm] + n]

            Eat = w16("Eat", CD)
            Eam = w16("Eam", PD, NAM)
            Wa2_98 = w16("Wa2_98", 98)
            Wa1_98 = w16("Wa1_98", 98)
            I16 = w16("I16", 128)
            E2 = w16("E2", 128, 33)
            Wc = w16("Wc", LAT)
            Wa = w16("Wa", LAT)
            Wg_flat = w16("Wg", H * GD)
            b98w = blobf[0:98, WFO["b98w"] : WFO["b98w"] + 1]
            b98r = blobf[0:98, WFO["b98r"] : WFO["b98r"] + 1]
            bc = blobf[0:LAT, WFO["bc"] : WFO["bc"] + 1]
            ba = blobf[0:LAT, WFO["ba"] : WFO["ba"] + 1]
            pw = blobf[0:LAT, WFO["pw"] : WFO["pw"] + 2]
            pb = blobf[0:1, WFO["pb"] : WFO["pb"] + 1]

            def MiT_v(lyr, i):
                off = W16O["MiT"] + (lyr * KW + i) * PD
                return blob16[:, off : off + PD]

            def Wgo_v(c):
                off = W16O["Wgo"] + c * CD
                return blob16[:, off : off + CD]

            def Wgoa4_v(c, s):
                off = W16O["Wgoa4"] + c * 4 + s
                return blob16[:, off : off + 2]

            I8 = blob8[:, 0:128]

            def MiT8_v(lyr, pr):
                off = 128 + (lyr * NPR + pr) * 2 * PD
                return bass.AP(tensor=blob8.tensor,
                               offset=blob8.offset + off,
                               ap=[blob8.ap[0], [PD, 2], [1, PD]])

            def leaky_act(out, in_, alpha, bias=None, accum_out=None):
                if use_prelu:
                    kw = {}
                    if bias is not None:
                        kw["bias"] = bias
                    if accum_out is not None:
                        kw["accum_out"] = accum_out
                    nc.scalar.activation(out=out, in_=in_, func=AF.Prelu,
                                         alpha=alpha, **kw)
                    return
                src = in_
                if bias is not None:
                    t = wpool.tile(list(out.shape), F32, tag="lk_t", bufs=4,
                                   name="lkb")
                    nc.scalar.activation(out=t, in_=in_, func=AF.Identity,
                                         bias=bias)
                    src = t
                nc.vector.scalar_tensor_tensor(
                    out=out, in0=src, scalar=alpha, in1=src,
                    op0=AT.mult, op1=AT.max)
                if accum_out is not None:
                    scr = wpool.tile(list(out.shape), F32, tag="lk_t2",
                                     bufs=4, name="lks")
                    nc.vector.tensor_scalar(out=scr, in0=out, scalar1=1.0,
                                            scalar2=0.0, op0=AT.mult,
                                            op1=AT.add, accum_out=accum_out)

            def dump_t(nm, src_ap, shape):
                if not dump:
                    return
                t = wpool.tile(shape, F32, tag="dumpbuf", bufs=2,
                               name="dump" + nm)
                nc.vector.tensor_copy(t, src_ap)
                dst = d_dbg[nm]
                sl = tuple([slice(None)] * len(shape))
                nc.sync.dma_start(out=dst[sl], in_=t)

            st = [dict() for _ in range(G)]

            # ================== phase 1: embeddings ==================
            for g in range(G):
                arow, prow, ladj8, amrow, pmrow = g_in[g]
                # atom one-hot -> avT [CD, N] (broadcast via PE)
                ab_ps = pssq.tile([128, N], F32, tag="sq", name="ab_ps")
                nc.tensor.matmul(ab_ps, onesrow, arow, start=True, stop=True)
                ohA = wpool.tile([128, N], MD, tag="t1k", bufs=6, name="ohA")
                nc.vector.tensor_tensor(out=ohA, in0=ab_ps, in1=iof[:, :N],
                                        op=AT.is_equal)
                avT_ps = pssq.tile([128, N], F32, tag="sq", name="avT_ps")
                nc.tensor.matmul(avT_ps, Eat, ohA, start=True, stop=True)
                avT = wpool.tile([128, N], MD, tag=f"avT{g}", bufs=1,
                                 name="avT")
                nc.scalar.copy(avT, avT_ps)
                st[g]["avT"] = avT

                # amino one-hot -> padded pv
                # dual-plane fp8 pv: plane1[k] = plane0[k+1] so DoubleRow
                # i-pairs read k-tiles at a 16B-aligned plane stride
                if CONV_FP8:
                    pv = apool.tile([PD, 2, PV8W], F8E4, tag=f"pv{g}_0",
                                    bufs=1, name="pv")
                    nc.vector.memset(pv[:, 0, :PADL], 0.0)
                    nc.vector.memset(pv[:, 0, PADL + L :], 0.0)
                    nc.vector.memset(pv[:, 1, : PADL - 1], 0.0)
                    nc.vector.memset(pv[:, 1, PADL - 1 + L :], 0.0)
                else:
                    pv = apool.tile([PD, PVW], MD, tag=f"pv{g}_0", bufs=1,
                                    name="pv")
                    nc.vector.memset(pv[:, :PADL], 0.0)
                    nc.vector.memset(pv[:, PADL + L :], 0.0)
                for c in range(2):
                    pb_ps = pssq.tile([128, N], F32, tag="sq", name="pb_ps")
                    nc.tensor.matmul(pb_ps, onesrow,
                                     prow[:, c * 512 : (c + 1) * 512],
                                     start=True, stop=True)
                    ohP = wpool.tile([NAM, N], MD, tag="t1k", bufs=6,
                                     name="ohP")
                    nc.vector.tensor_tensor(
                        out=ohP, in0=pb_ps[:NAM, :],
                        in1=iof[:NAM, c * 512 : (c + 1) * 512],
                        op=AT.is_equal)
                    pvT_ps = pscv.tile([PD, 512], F32, tag="cv",
                                       name="pvT_ps")
                    nc.tensor.matmul(pvT_ps, Eam, ohP, start=True, stop=True)
                    if CONV_FP8:
                        nc.scalar.copy(
                            pv[:, 0, PADL + c * 512 : PADL + (c + 1) * 512],
                            pvT_ps)
                        nc.vector.tensor_copy(
                            pv[:, 1, PADL - 1 + c * 512 :
                               PADL - 1 + (c + 1) * 512], pvT_ps)
                    else:
                        nc.scalar.copy(
                            pv[:, PADL + c * 512 : PADL + (c + 1) * 512],
                            pvT_ps)
                st[g]["pv"] = pv

            # ================== phase 2: per-graph prep ==================
            for g in range(G):
                avT = st[g]["avT"]
                # all heads' Wh -> whsb_all[p, t, h, 0:64]; col 64 = 1.0
                whsb_all = wpool.tile([128, NT, H, GD + 2], MD,
                                      tag=f"whsb{g}", bufs=1, name="whsb_all")
                nc.vector.memset(whsb_all[:, :, :, GD : GD + 1], 1.0)
                for half in range(2):
                    wh_all = pssq.tile([128, 2, H * GD], F32, tag="sq",
                                       name="wh_all")
                    for t2 in range(2):
                        t = half * 2 + t2
                        nc.tensor.matmul(
                            wh_all[:, t2, :],
                            avT[:, t * 128 : (t + 1) * 128],
                            Wg_flat, start=True, stop=True)
                    nc.scalar.copy(
                        out=bass.AP(
                            tensor=whsb_all.tensor,
                            offset=whsb_all.offset
                            + half * 2 * H * (GD + 2),
                            ap=[whsb_all.ap[0], [H * (GD + 2), 2],
                                [GD + 2, H], [1, GD]]),
                        in_=wh_all)
                st[g]["whsb"] = whsb_all

                # z operands: zws[32h]=dst_h, [32h+1]=1; zrs[32h]=1,
                # [32h+1]=src_h  (heads at legal partition starts)
                zw_t = pssq.tile([128, N], F32, tag="sq", name="zw_ps")
                zw_ps = zw_t[0:98, :]
                nc.tensor.matmul(zw_ps, Wa2_98, avT, start=True, stop=True)
                zws = wpool.tile([98, N], MD, tag=f"zws{g}", bufs=1,
                                 name="zws")
                nc.scalar.activation(out=zws, in_=zw_ps, func=AF.Identity,
                                     bias=b98w)
                zr_t = pssq.tile([128, N], F32, tag="sq", name="zr_ps")
                zr_ps = zr_t[0:98, :]
                nc.tensor.matmul(zr_ps, Wa1_98, avT, start=True, stop=True)
                zrs = wpool.tile([98, N], MD, tag=f"zrs{g}", bufs=1,
                                 name="zrs")
                nc.scalar.activation(out=zrs, in_=zr_ps, func=AF.Identity,
                                     bias=b98r)
                st[g]["zws"] = zws
                st[g]["zrs"] = zrs

            # ================== conv machinery ==================
            conv_steps = []

            def make_conv_layer(lyr):
                pvo_l, cv = [], {}
                last = lyr == LC - 1
                for g in range(G):
                    if last:
                        pvo = apool.tile([PD, PVW], MD, tag=f"pvf{g}",
                                         bufs=1, name="pvo")
                    elif CONV_FP8:
                        pvo = apool.tile([PD, 2, PV8W], F8E4,
                                         tag=f"pv{g}_{1 - lyr % 2}", bufs=1,
                                         name="pvo")
                    else:
                        pvo = apool.tile([PD, PVW], MD,
                                         tag=f"pv{g}_{1 - lyr % 2}", bufs=1,
                                         name="pvo")
                    pvo_l.append(pvo)
                steps = []

                def mk_group(g, c):
                    def run():
                        if c == 0:
                            if last or not CONV_FP8:
                                nc.vector.memset(pvo_l[g][:, :PADL], 0.0)
                                nc.vector.memset(pvo_l[g][:, PADL + L :],
                                                 0.0)
                            else:
                                nc.vector.memset(pvo_l[g][:, 0, :PADL], 0.0)
                                nc.vector.memset(
                                    pvo_l[g][:, 0, PADL + L :], 0.0)
                                nc.vector.memset(
                                    pvo_l[g][:, 1, : PADL - 1], 0.0)
                                nc.vector.memset(
                                    pvo_l[g][:, 1, PADL - 1 + L :], 0.0)
                        cv[g] = pscv.tile([PD, 512], F32, tag="cv",
                                          name=f"cv{g}")
                        pv = st[g]["pv"]
                        if CONV_FP8:
                            for pr in range(NPR):
                                nc.tensor.matmul(
                                    cv[g], MiT8_v(lyr, pr),
                                    bass.AP(tensor=pv.tensor,
                                            offset=pv.offset + c * 512
                                            + 2 * pr,
                                            ap=[pv.ap[0], [PV8W, 2],
                                                [1, 512]]),
                                    start=(pr == 0), stop=(pr == NPR - 1),
                                    perf_mode=DR)
                        else:
                            for i in range(KW):
                                nc.tensor.matmul(
                                    cv[g], MiT_v(lyr, i),
                                    pv[:, c * 512 + i : c * 512 + i + 512],
                                    start=(i == 0), stop=(i == KW - 1))
                    return run

                def mk_relu(g, c):
                    def run():
                        if last or not CONV_FP8:
                            nc.vector.tensor_scalar(
                                out=pvo_l[g][:, PADL + c * 512 :
                                             PADL + (c + 1) * 512],
                                in0=cv[g], scalar1=cb[:, lyr : lyr + 1],
                                scalar2=0.0, op0=AT.add, op1=AT.max)
                        else:
                            nc.vector.tensor_scalar(
                                out=pvo_l[g][:, 0, PADL + c * 512 :
                                             PADL + (c + 1) * 512],
                                in0=cv[g], scalar1=cb[:, lyr : lyr + 1],
                                scalar2=0.0, op0=AT.add, op1=AT.max)
                            nc.vector.tensor_scalar(
                                out=pvo_l[g][:, 1, PADL - 1 + c * 512 :
                                             PADL - 1 + (c + 1) * 512],
                                in0=cv[g], scalar1=cb[:, lyr : lyr + 1],
                                scalar2=0.0, op0=AT.add, op1=AT.max)
                    return run

                for c in range(2):
                    for g in range(G):
                        steps.append(mk_group(g, c))
                        steps.append(mk_relu(g, c))

                def finish():
                    for g in range(G):
                        st[g]["pv"] = pvo_l[g]
                    if dump and lyr == LC - 1:
                        dump_t("pv3", pvo_l[0], [128, PVW])
                steps.append(finish)
                return steps

            for lyr in range(LC):
                conv_steps.extend(make_conv_layer(lyr))

            def pop_conv(k):
                for _ in range(k):
                    if conv_steps:
                        conv_steps.pop(0)()

            # ============ attention z/U (per head or out-layer) ============
            map_idx = [0]

            def att_zU(g, zw2, zr2, dump_zu=False, tp=None):
                """z on PE -> leaky -> exp. zw2/zr2: [2, N] slices
                ([dst;1], [1;src]). Returns U [128, NT, N] fp16."""
                ladj8 = g_in[g][2]
                ee = apool.tile([128, NT, N], MD, tag="ee", bufs=2, name="ee")
                map_idx[0] += 1
                for th in range(2):
                    zps = psz.tile([128, 2, N], F32, tag="z", name="zps")
                    for t2 in range(2):
                        t = th * 2 + t2
                        zc = zps[:, t2, :]
                        nc.tensor.matmul(zc, I8, ladj8[:, t, :],
                                         start=True, stop=False)
                        nc.tensor.matmul(
                            zc, zw2[:, t * 128 : (t + 1) * 128], zr2,
                            start=False, stop=True, tile_position=tp)
                    sl = slice(th * 2, th * 2 + 2)
                    if use_prelu:
                        nc.scalar.activation(out=ee[:, sl, :], in_=zps,
                                             func=AF.Prelu, alpha=ALPHA)
                    else:
                        eesc = wpool.tile([128, 2, N], F32, tag="eesc",
                                          bufs=2, name="eesc")
                        nc.vector.tensor_copy(eesc, zps)
                        nc.vector.scalar_tensor_tensor(
                            out=ee[:, sl, :], in0=eesc, scalar=ALPHA,
                            in1=eesc, op0=AT.mult, op1=AT.max)
                if dump_zu:
                    dump_t("zm0", ee, [128, NT, N])
                U = apool.tile([128, NT, N], MD, tag="U", bufs=2, name="U")
                nc.scalar.activation(out=U, in_=ee, func=AF.Exp)
                if dump_zu:
                    dump_t("U0", U, [128, NT, N])
                return U

            def elu_norm_pair(hp0, hp1, dst):
                """Normalize two heads' hp [65, N] (row 64 = rowsum) and
                elu into packed dst [128, N]."""
                zrw2 = rpool.tile([33, N], F32, tag="zrw2", bufs=3,
                                  name="zrw2")
                nc.gpsimd.memset(zrw2, 1.0)
                nc.vector.tensor_copy(zrw2[0:1, :], hp0[64:65, :])
                nc.vector.tensor_copy(zrw2[32:33, :], hp1[64:65, :])
                rinv2 = rpool.tile([33, N], F32, tag="zrw2", bufs=3,
                                   name="rinv2")
                scr = rpool.tile([33, N], F32, tag="zrw2", bufs=3,
                                 name="rscr")
                nc.vector.reciprocal_approx_accurate(out=rinv2, in_=zrw2,
                                                     scratch=scr)
                rinv16 = rpool.tile([33, N], MD, tag="ri16", bufs=2,
                                    name="rinv16")
                nc.vector.tensor_scalar(out=rinv16, in0=rinv2, scalar1=1.0,
                                        scalar2=None, op0=AT.mult)
                rb_ps = pssq.tile([128, N], F32, tag="sq", name="rb_ps")
                nc.tensor.matmul(rb_ps, E2, rinv16, start=True, stop=True)
                rb = wpool.tile([128, N], MD, tag="rb", bufs=2, name="rb")
                nc.vector.tensor_copy(rb, rb_ps)
                hpn = wpool.tile([128, N], MD, tag="elu", bufs=4, name="hpn")
                nc.vector.scalar_tensor_tensor(
                    out=hpn[0:64, :], in0=hp0[:64, :], scalar=1.0,
                    in1=rb[0:64, :], op0=AT.mult, op1=AT.mult)
                nc.vector.scalar_tensor_tensor(
                    out=hpn[64:128, :], in0=hp1[:64, :], scalar=1.0,
                    in1=rb[64:128, :], op0=AT.mult, op1=AT.mult)
                xm = wpool.tile([128, N], MD, tag="elu", bufs=4, name="xm")
                nc.vector.tensor_scalar(out=xm, in0=hpn, scalar1=0.0,
                                        scalar2=None, op0=AT.min)
                em = wpool.tile([128, N], MD, tag="elu", bufs=4, name="em")
                nc.scalar.activation(out=em, in_=xm, func=AF.Exp)
                nc.vector.scalar_tensor_tensor(
                    out=dst, in0=em, scalar=-1.0, in1=hpn,
                    op0=AT.add, op1=AT.max)

            # ================== phase 3: head maps + conv ==================
            for g in range(G):
                m01 = wpool.tile([128, N], MD, tag=f"m01{g}", bufs=1,
                                 name="m01")
                m23 = wpool.tile([128, N], MD, tag=f"m23{g}", bufs=1,
                                 name="m23")
                st[g]["multi"] = (m01, m23)
                st[g]["hp"] = {}

            # software-pipelined attention: emit z/U of map k+1 before the
            # hp matmuls of map k so the PE never stalls on the exp chain
            pend = None          # (g, h, U) waiting for its hp emission
            pair_hps = {}

            def emit_hp(g, h, U):
                whsb_all = st[g]["whsb"]
                hp_t = pshp.tile([128, N], F32, tag="hp", name="hp_ps")
                hp = hp_t[0:65, :]
                for t in range(NT):
                    nc.tensor.matmul(hp, whsb_all[:, t, h, : GD + 1],
                                     U[:, t, :],
                                     start=(t == 0), stop=(t == NT - 1))
                pair_hps.setdefault(g, []).append(hp)
                if len(pair_hps[g]) == 2:
                    hps = pair_hps.pop(g)
                    elu_norm_pair(hps[0], hps[1], st[g]["multi"][h // 2])

            # warm-up: conv L1 as one dense block before the attention maps
            pop_conv(8)
            for hp_i in range(H // 2):
                for g in range(G):
                    zws, zrs = st[g]["zws"], st[g]["zrs"]
                    for h2 in range(2):
                        h = hp_i * 2 + h2
                        U = att_zU(g, zws[32 * h : 32 * h + 2, :],
                                   zrs[32 * h : 32 * h + 2, :],
                                   dump_zu=(dump and g == 0 and h == 0),
                                   tp=(32 * h, 0))
                        if pend is not None:
                            emit_hp(*pend)
                        pend = (g, h, U)
                        pop_conv(1)
            if pend is not None:
                emit_hp(*pend)
                pend = None

            if dump:
                dump_t("m01", st[0]["multi"][0], [128, N])
                dump_t("m23", st[0]["multi"][1], [128, N])

            # ================== phase 4: GAT output layer ==================
            for g in range(G):
                m01, m23 = st[g]["multi"]
                multi = [m01, m23]
                wh2_ps = pssq.tile([128, NT, CD], F32, tag="sq",
                                   name="wh2_ps")
                for t in range(NT):
                    for c in range(2):
                        nc.tensor.matmul(
                            wh2_ps[:, t, :],
                            multi[c][:, t * 128 : (t + 1) * 128],
                            Wgo_v(c), start=(c == 0), stop=(c == 1))
                wh2sb = wpool.tile([128, NT, CD], MD, tag=f"wh2{g}", bufs=1,
                                   name="wh2sb")
                nc.vector.tensor_copy(wh2sb, wh2_ps)
                st[g]["wh2sb"] = wh2sb

                zw2_t = pssq.tile([128, N], F32, tag="sq", name="zw2_ps")
                zw2_ps = zw2_t[0:2, :]
                for c in range(2):
                    nc.tensor.matmul(zw2_ps, Wgoa4_v(c, 0), multi[c],
                                     start=(c == 0), stop=(c == 1))
                zw2 = rpool.tile([2, N], MD, tag="zw2", bufs=4, name="zw2")
                b2w = rpool.tile([2, 1], F32, tag="b2", bufs=4, name="b2w")
                nc.vector.memset(b2w, 1.0)
                nc.vector.memset(b2w[0:1, :], 0.0)
                nc.scalar.activation(out=zw2, in_=zw2_ps, func=AF.Identity,
                                     bias=b2w)
                zr2_t = pssq.tile([128, N], F32, tag="sq", name="zr2_ps")
                zr2_ps = zr2_t[0:2, :]
                for c in range(2):
                    nc.tensor.matmul(zr2_ps, Wgoa4_v(c, 2), multi[c],
                                     start=(c == 0), stop=(c == 1))
                zr2 = rpool.tile([2, N], MD, tag="zw2", bufs=4, name="zr2")
                b2r = rpool.tile([2, 1], F32, tag="b2", bufs=4, name="b2r")
                nc.vector.memset(b2r, 0.0)
                nc.vector.memset(b2r[0:1, :], 1.0)
                nc.scalar.activation(out=zr2, in_=zr2_ps, func=AF.Identity,
                                     bias=b2r)
                st[g]["zw2"] = zw2
                st[g]["zr2"] = zr2
                pop_conv(3)

            oU, ohp, ors, orinv, orb = {}, {}, {}, {}, {}
            for g in range(G):
                oU[g] = att_zU(g, st[g]["zw2"], st[g]["zr2"])
                pop_conv(1)
            for g in range(G):
                wh2sb = st[g]["wh2sb"]
                hp2 = pshp.tile([128, N], F32, tag="hp", name="hp2_ps")
                for t in range(NT):
                    nc.tensor.matmul(hp2, wh2sb[:, t, :], oU[g][:, t, :],
                                     start=(t == 0), stop=(t == NT - 1))
                rs_t = psz.tile([128, 2, N], F32, tag="z", name="rs_ps")
                rs_ps = rs_t[:, 0, :]
                for t in range(NT):
                    nc.tensor.matmul(rs_ps[0:1, :], ones16c[:, 0:1],
                                     oU[g][:, t, :],
                                     start=(t == 0), stop=(t == NT - 1))
                ohp[g] = hp2
                ors[g] = rs_ps
                pop_conv(1)
            for g in range(G):
                zrw = rpool.tile([1, N], F32, tag="zrwo", bufs=6, name="zrw")
                nc.scalar.copy(zrw, ors[g][0:1, :])
                rinv = rpool.tile([1, N], F32, tag="zrwo", bufs=6,
                                  name="rinv")
                scr = rpool.tile([1, N], F32, tag="zrwo", bufs=6,
                                 name="rscr")
                nc.vector.reciprocal_approx_accurate(out=rinv, in_=zrw,
                                                     scratch=scr)
                if dump and g == 0:
                    dump_t("rinv0", rinv, [1, N])
                rinv16 = rpool.tile([1, N], MD, tag="ri16o", bufs=2,
                                    name="rinv16o")
                nc.vector.tensor_scalar(out=rinv16, in0=rinv, scalar1=1.0,
                                        scalar2=None, op0=AT.mult)
                orinv[g] = rinv16
            for g in range(G):
                rb_ps = pssq.tile([128, N], F32, tag="sq", name="rb2_ps")
                nc.tensor.matmul(rb_ps, onesrow, orinv[g], start=True,
                                 stop=True)
                rb = wpool.tile([128, N], MD, tag="rb", bufs=2, name="rbo")
                nc.vector.tensor_copy(rb, rb_ps)
                orb[g] = rb
            for g in range(G):
                xT = wpool.tile([CD, N], MD, tag=f"xT{g}", bufs=1, name="xT")
                hpn = wpool.tile([128, N], MD, tag="elu", bufs=4,
                                 name="hpno")
                nc.vector.scalar_tensor_tensor(
                    out=hpn, in0=ohp[g], scalar=1.0, in1=orb[g],
                    op0=AT.mult, op1=AT.mult)
                xm = wpool.tile([128, N], MD, tag="elu", bufs=4, name="xmo")
                nc.vector.tensor_scalar(out=xm, in0=hpn, scalar1=0.0,
                                        scalar2=None, op0=AT.min)
                em = wpool.tile([128, N], MD, tag="elu", bufs=4, name="emo")
                nc.scalar.activation(out=em, in_=xm, func=AF.Exp)
                nc.vector.scalar_tensor_tensor(
                    out=xT, in0=em, scalar=-1.0, in1=hpn,
                    op0=AT.add, op1=AT.max)
                st[g]["xT"] = xT
                if dump and g == 0:
                    dump_t("xT", xT, [128, N])
                pop_conv(2)

            # ============ phase 5: comp head + conv (graph-stepped) ========
            oav, oavec, oav2 = {}, {}, {}
            for g in range(G):
                av_t = pshp.tile([128, N], F32, tag="hp", name="av_ps")
                oav[g] = av_t[:LAT, :]
                nc.tensor.matmul(oav[g], Wc, st[g]["xT"], start=True,
                                 stop=True)
                pop_conv(1)
            for g in range(G):
                avec = wpool.tile([LAT, N], MD, tag="avec", bufs=2,
                                  name="avec")
                leaky_act(avec, oav[g], ALPHA, bias=bc)
                oavec[g] = avec
            for g in range(G):
                av2_t = pshp.tile([128, N], F32, tag="hp", name="av2_ps")
                oav2[g] = av2_t[:LAT, :]
                nc.tensor.matmul(oav2[g], Wa, oavec[g], start=True,
                                 stop=True)
                pop_conv(1)
            for g in range(G):
                a_v = wpool.tile([LAT, N], MD, tag="avec", bufs=2,
                                 name="a_v")
                comp_acc = rpool.tile([LAT, 1], F32, tag="c1", bufs=8,
                                      name="comp_acc")
                leaky_act(a_v, oav2[g], ALPHA, bias=ba, accum_out=comp_acc)
                st[g]["comp_acc"] = comp_acc
            for g in range(G):
                comp_acc = st[g]["comp_acc"]
                amrow = g_in[g][3]
                amscr = rpool.tile([1, N], F32, tag="r2k", bufs=6,
                                   name="amscr")
                amsum = rpool.tile([1, 1], F32, tag="c2", bufs=12,
                                   name="amsum")
                nc.vector.tensor_scalar(out=amscr, in0=amrow, scalar1=1.0,
                                        scalar2=0.0, op0=AT.mult, op1=AT.add,
                                        accum_out=amsum)
                amsb = rpool.tile([128, 1], F32, tag="c2", bufs=12,
                                  name="amsb")
                nc.gpsimd.partition_broadcast(amsb, amsum)
                amr = rpool.tile([128, 1], F32, tag="c2", bufs=12,
                                 name="amr")
                nc.vector.reciprocal(amr, amsb)
                cp = rpool.tile([128, 2], F32, tag="cp", bufs=4, name="cp")
                nc.vector.tensor_scalar(out=cp[:, 0:1], in0=comp_acc,
                                        scalar1=amr, scalar2=None,
                                        op0=AT.mult)
                st[g]["cp"] = cp
                if dump and g == 0:
                    dump_t("comp", comp_acc, [LAT, 1])
                pop_conv(3)

            pop_conv(len(conv_steps))

            # ========= phase 6: prot head + prediction (graph-stepped) =====
            opacc = {}
            for c in range(2):
                for g in range(G):
                    pv = st[g]["pv"]
                    pvt = psz.tile([128, 2, N], F32, tag="z", name="pv_ps")
                    pv_ps = pvt[:LAT, 0, :]
                    nc.tensor.matmul(pv_ps, Wa,
                                     pv[:, PADL + c * 512 :
                                        PADL + (c + 1) * 512],
                                     start=True, stop=True)
                    p_v = wpool.tile([LAT, 512], MD, tag="p_v", bufs=4,
                                     name="p_v")
                    pacc = rpool.tile([LAT, 1], F32, tag="c1", bufs=8,
                                      name="pacc")
                    leaky_act(p_v, pv_ps, ALPHA, bias=ba, accum_out=pacc)
                    opacc.setdefault(g, []).append(pacc)
            for g in range(G):
                cp = st[g]["cp"]
                prot_acc = rpool.tile([LAT, 1], F32, tag="c1", bufs=8,
                                      name="prot_acc")
                nc.vector.tensor_tensor(out=prot_acc, in0=opacc[g][0],
                                        in1=opacc[g][1], op=AT.add)
                if dump and g == 0:
                    dump_t("prot", prot_acc, [LAT, 1])

                pmrow = g_in[g][4]
                pmscr = rpool.tile([1, L], F32, tag="r4k", bufs=2,
                                   name="pmscr")
                pmsum = rpool.tile([1, 1], F32, tag="c2", bufs=12,
                                   name="pmsum")
                nc.vector.tensor_scalar(out=pmscr, in0=pmrow, scalar1=1.0,
                                        scalar2=0.0, op0=AT.mult, op1=AT.add,
                                        accum_out=pmsum)
                pmsb = rpool.tile([128, 1], F32, tag="c2", bufs=12,
                                  name="pmsb")
                nc.gpsimd.partition_broadcast(pmsb, pmsum)
                pmr = rpool.tile([128, 1], F32, tag="c2", bufs=12, name="pmr")
                nc.vector.reciprocal(pmr, pmsb)
                nc.vector.tensor_scalar(out=cp[:, 1:2], in0=prot_acc,
                                        scalar1=pmr, scalar2=None,
                                        op0=AT.mult)

                lr2 = rpool.tile([128, 2], F32, tag="cp", bufs=4, name="lr2")
                leaky_act(lr2, cp, ALPHA * ALPHA)
                dscr = rpool.tile([128, 2], F32, tag="cp", bufs=4,
                                  name="dscr")
                dacc = rpool.tile([128, 1], F32, tag="c1", bufs=8,
                                  name="dacc")
                nc.vector.scalar_tensor_tensor(
                    out=dscr, in0=lr2, scalar=1.0, in1=pw,
                    op0=AT.mult, op1=AT.mult, accum_out=dacc)
                fin_ps = pssq.tile([128, N], F32, tag="sq", name="fin_ps")
                nc.tensor.matmul(fin_ps[0:1, 0:1], dacc, ones_col,
                                 start=True, stop=True)
                res = rpool.tile([1, 1], F32, tag="c2", bufs=12, name="res")
                nc.scalar.activation(out=res, in_=fin_ps[0:1, 0:1],
                                     func=AF.Identity, bias=pb)
                nc.sync.dma_start(out=d_out[g : g + 1, :], in_=res)

    return nc


def preprocess(inputs, mm_bf16=False):
    """Host-side prep: shard over cores, transpose/reshape weights."""
    import ml_dtypes
    md = np.float16
    f8 = ml_dtypes.float8_e5m2
    atoms = np.asarray(inputs["atoms"]).astype(np.float32)
    atoms_mask = np.asarray(inputs["atoms_mask"]).astype(np.float32)
    adjacency = np.asarray(inputs["adjacency"])
    amino = np.asarray(inputs["amino"]).astype(np.float32)
    amino_mask = np.asarray(inputs["amino_mask"]).astype(np.float32)
    E_atom = np.asarray(inputs["E_atom"]).astype(np.float32)
    E_amino = np.asarray(inputs["E_amino"]).astype(np.float32)
    W_gat = np.asarray(inputs["W_gat"]).astype(np.float32)
    a_gat = np.asarray(inputs["a_gat"]).astype(np.float32)
    W_go = np.asarray(inputs["W_go"]).astype(np.float32)
    a_go = np.asarray(inputs["a_go"]).astype(np.float32)
    W_comp_w = np.asarray(inputs["W_comp_w"]).astype(np.float32)
    W_comp_b = np.asarray(inputs["W_comp_b"]).astype(np.float32)
    conv_w = np.asarray(inputs["conv_w"]).astype(np.float32)
    conv_b = np.asarray(inputs["conv_b"]).astype(np.float32)
    W_att_w = np.asarray(inputs["W_att_w"]).astype(np.float32)
    W_att_b = np.asarray(inputs["W_att_b"]).astype(np.float32)
    pred_w = np.asarray(inputs["pred_w"]).astype(np.float32)
    pred_b = np.asarray(inputs["pred_b"]).astype(np.float32)

    ladjT = np.where(adjacency.transpose(0, 2, 1) > 0, np.float32(0.0),
                     np.float32(MASKNEG)).astype(np.float32)
    ladjT_r = np.ascontiguousarray(
        ladjT.reshape(B, NT, 128, N).transpose(0, 2, 1, 3))

    E_atom_pad = np.zeros((128, CD), np.float32)
    E_atom_pad[:NA] = E_atom

    MiT = np.zeros((LC, KW, PD, PD), np.float32)
    din = np.arange(PD)[:, None]
    dout = np.arange(PD)[None, :]
    v = din - dout + (KW // 2)
    valid = (v >= 0) & (v < KW)
    vc = np.clip(v, 0, KW - 1)
    for lyr in range(LC):
        for i in range(KW):
            MiT[lyr, i] = np.where(valid, conv_w[lyr, 0, 0, i, vc], 0.0)
    MiT_r = np.ascontiguousarray(MiT.transpose(2, 0, 1, 3))
    NPRl = (KW + 1) // 2
    MiT8 = np.zeros((LC, NPRl, 2, PD, PD), np.float32)
    for lyr in range(LC):
        for pr in range(NPRl):
            MiT8[lyr, pr, 0] = MiT[lyr, 2 * pr]
            if 2 * pr + 1 < KW:
                MiT8[lyr, pr, 1] = MiT[lyr, 2 * pr + 1]
    MiT8_r = np.ascontiguousarray(MiT8.transpose(3, 0, 1, 2, 4))

    W_gat_r = np.ascontiguousarray(W_gat.transpose(1, 0, 2))
    Wa1h = np.einsum("hpq,hq->ph", W_gat, a_gat[:, :GD])
    Wa2h = np.einsum("hpq,hq->ph", W_gat, a_gat[:, GD:])
    Wa2_98 = np.zeros((CD, 98), np.float32)
    Wa2_98[:, [0, 32, 64, 96]] = Wa2h
    Wa1_98 = np.zeros((CD, 98), np.float32)
    Wa1_98[:, [1, 33, 65, 97]] = Wa1h
    b98w = np.zeros((98, 1), np.float32)
    b98w[[1, 33, 65, 97]] = 1.0
    b98r = np.zeros((98, 1), np.float32)
    b98r[[0, 32, 64, 96]] = 1.0

    W_go_r = np.ascontiguousarray(
        W_go.reshape(2, 128, CD).transpose(1, 0, 2))
    Wgo_a2 = (W_go @ a_go[CD:]).reshape(2, 128).T     # (128, 2)
    Wgo_a1 = (W_go @ a_go[:CD]).reshape(2, 128).T
    Wgoa4 = np.zeros((128, 2, 4), np.float32)
    Wgoa4[:, :, 0] = Wgo_a2
    Wgoa4[:, :, 3] = Wgo_a1

    I8dr = np.zeros((128, 2, 128), np.float32)
    I8dr[:, 0, :] = np.eye(128, dtype=np.float32)
    I16 = np.eye(128, dtype=np.float32)
    E2 = np.zeros((33, 128), np.float32)
    E2[0, 0:64] = 1.0
    E2[32, 64:128] = 1.0

    # pack fp16 weight blob (offsets mirror kernel W16O)
    W16N = 1484 + LC * KW * PD
    blob16 = np.zeros((128, W16N), np.float32)

    def put16(off, arr):
        a = np.asarray(arr, np.float32)
        blob16[: a.shape[0], off : off + a.shape[1]] = a

    put16(0, E_atom_pad)
    put16(128, E_amino)
    put16(256, W_gat_r.reshape(CD, H * GD))
    put16(512, Wa2_98)
    put16(610, Wa1_98)
    put16(708, W_go_r.reshape(128, 2 * CD))
    put16(964, Wgoa4.reshape(128, 8))
    put16(972, I16)
    put16(1100, E2)
    put16(1228, W_comp_w.T)
    put16(1356, W_att_w.T)
    put16(1484, MiT_r.reshape(PD, LC * KW * PD))

    blobf = np.zeros((128, 7), np.float32)
    blobf[:98, 0] = b98w[:, 0]
    blobf[:98, 1] = b98r[:, 0]
    blobf[:LAT, 2] = W_comp_b
    blobf[:LAT, 3] = W_att_b
    blobf[:LAT, 4] = pred_w[0, :LAT]
    blobf[:LAT, 5] = pred_w[0, LAT:]
    blobf[0, 6] = pred_b[0]

    blob8 = np.zeros((128, 128 + LC * NPRl * 2 * PD), np.float32)
    blob8[:, :128] = I16
    blob8[:, 128:] = MiT8_r.reshape(PD, LC * NPRl * 2 * PD)

    shared = {
        "blob16": blob16.astype(md),
        "blobf": blobf,
        "blob8": blob8.astype(ml_dtypes.float8_e4m3fn),
        "conv_b": np.ascontiguousarray(conv_b.reshape(LC, 1)),
    }
    in_maps = []
    for c in range(NCORES):
        sl = slice(c * G, (c + 1) * G)
        m = dict(shared)
        m["atoms_f"] = np.ascontiguousarray(atoms[sl]).astype(md)
        m["atoms_mask"] = np.ascontiguousarray(atoms_mask[sl])
        m["ladj8"] = np.ascontiguousarray(ladjT_r[sl]).astype(f8)
        m["amino_f"] = np.ascontiguousarray(amino[sl]).astype(md)
        m["amino_mask"] = np.ascontiguousarray(amino_mask[sl])
        in_maps.append(m)
    return in_maps


_CACHED_NC = None


def kernel(**inputs) -> np.ndarray:
    global _CACHED_NC
    from concourse.bass_utils import run_bass_kernel_spmd

    if _CACHED_NC is None:
        nc = build_core_program()
        nc.finalize()
        _CACHED_NC = nc
    nc = _CACHED_NC
    in_maps = preprocess(inputs)
    res = run_bass_kernel_spmd(nc, in_maps, core_ids=list(range(NCORES)))
    out = np.concatenate([res.results[c]["out"] for c in range(NCORES)], axis=0)
    return out.astype(np.float32)


# revision 36
# speedup vs baseline: 1.0004x; 1.0004x over previous
"""Trainium2 Bass kernel for BiDACPI (GAT + CNN + bidirectional attention).

Data-parallel over batch b=16 across 8 NeuronCores (2 graphs per core).
Self-contained: hardcodes all shapes; host-side preprocessing only reshapes /
transposes weights and converts index tensors.

v3: attention logits z = mask + src_i + dst_j are built ON THE PE
(fp8-DoubleRow identity x mask matmul + K=2 rank-2 matmul of
[dst;ones]^T [ones;src]), eliminating the DVE broadcast/accumulator ops
that dominated v2. Heads are processed in pairs so softmax-normalize +
elu run on packed [128, 512] tiles. fp16 matmuls; f32 on DVE
scalar_tensor_tensor paths (fp16 is slower there); fp16 only where DVE
tensor_scalar/copy 4x modes apply. Pooling is folded into activation
accum_out (masks are spec'd fill=ones).
"""
import numpy as np

import concourse.bass as bass
import concourse.mybir as mybir
import concourse.tile as tile
from concourse import bacc

F32 = mybir.dt.float32
F16 = mybir.dt.float16
F8 = mybir.dt.float8e5
F8E4 = mybir.dt.float8e4
I32 = mybir.dt.int32
AT = mybir.AluOpType
AF = mybir.ActivationFunctionType
DR = mybir.MatmulPerfMode.DoubleRow

# Problem constants
B = 16
NCORES = 8
G = B // NCORES          # graphs per core
N = 512                  # atoms per graph
L = 1024                 # amino length
CD = 128                 # comp_dim
PD = 128                 # prot_dim
GD = 64                  # gat_dim
H = 4                    # heads
LAT = 128                # latent
NA = 100                 # num_atom
NAM = 30                 # num_amino
LC = 3                   # conv layers
KW = 11                  # conv kernel width
ALPHA = 0.2
MASKNEG = -28672.0       # fp8e5-exact additive mask
NT = N // 128            # 4 j-chunks
PADL = KW // 2
PVW = PADL + L + PADL + 2  # padded pv width (1036, even)

MASK_FP8_DR = True       # fp8e5 identity-mask matmul
LEAKY_SPLIT = 5          # unused
CONV_FP8 = False          # conv band matmuls in fp8e4m3 + DoubleRow i-pairs
NPR = (KW + 1) // 2      # 6 DoubleRow pairs
PV8W = 1040              # fp8 dual-plane width (16B-aligned plane stride)


def build_core_program(debug=False, mm_bf16=False, dump=False):
    """debug=True builds the CoreSim-compatible variant (no Prelu; no
    activation accum_out)."""
    if debug:
        nc = bacc.Bacc(None, target_bir_lowering=False, debug=True)
    else:
        nc = bacc.Bacc(None)
    MD = F16
    use_prelu = not debug

    # ---- DRAM I/O ----
    d_atoms = nc.dram_tensor("atoms_f", [G, N], MD, kind="ExternalInput")
    d_amask = nc.dram_tensor("atoms_mask", [G, N], F32, kind="ExternalInput")
    # ladj8[g, p, t, i] = additive mask for edge j=t*128+p -> i (fp8e5)
    d_ladj8 = nc.dram_tensor("ladj8", [G, 128, NT, N], F8,
                             kind="ExternalInput")
    d_amino = nc.dram_tensor("amino_f", [G, L], MD, kind="ExternalInput")
    d_pmask = nc.dram_tensor("amino_mask", [G, L], F32, kind="ExternalInput")
    # packed weight blobs (one DMA each): offsets must match preprocess()
    W16O = dict(Eat=0, Eam=128, Wg=256, Wa2_98=512, Wa1_98=610, Wgo=708,
                Wgoa4=964, I16=972, E2=1100, Wc=1228, Wa=1356,
                MiT=1484)
    W16N = 1484 + LC * KW * PD
    WFO = dict(b98w=0, b98r=1, bc=2, ba=3, pw=4, pb=6)
    WFN = 7
    d_blob16 = nc.dram_tensor("blob16", [128, W16N], MD,
                              kind="ExternalInput")
    d_blobf = nc.dram_tensor("blobf", [128, WFN], F32, kind="ExternalInput")
    d_blob8 = nc.dram_tensor("blob8", [128, 128 + LC * NPR * 2 * PD], F8E4,
                             kind="ExternalInput")
    d_cb = nc.dram_tensor("conv_b", [LC, 1], F32, kind="ExternalInput")
    d_out = nc.dram_tensor("out", [G, 1], F32, kind="ExternalOutput")
    d_dbg = {}
    if dump:
        for nm, shp in [("U0", [128, NT, N]), ("zm0", [128, NT, N]),
                        ("m01", [128, N]), ("m23", [128, N]),
                        ("xT", [128, N]), ("rinv0", [1, N]),
                        ("pv3", [128, PVW]), ("comp", [LAT, 1]),
                        ("prot", [LAT, 1])]:
            d_dbg[nm] = nc.dram_tensor("dbg_" + nm, shp, F32,
                                       kind="ExternalOutput")

    with tile.TileContext(nc) as tc:
        with (
            tc.tile_pool(name="const", bufs=1) as cpool,
            tc.tile_pool(name="work", bufs=1) as wpool,
            tc.tile_pool(name="att", bufs=1) as apool,
            tc.tile_pool(name="rows", bufs=1) as rpool,
            tc.tile_pool(name="ps_sq", bufs=1, space="PSUM") as pssq,
            tc.tile_pool(name="ps_hp", bufs=2, space="PSUM") as pshp,
            tc.tile_pool(name="ps_z", bufs=2, space="PSUM") as psz,
            tc.tile_pool(name="ps_cv", bufs=1, space="PSUM") as pscv,
        ):
            # queue heads: iota first on gpsimd (gates one-hots), const
            # memsets first on DVE
            ioi = cpool.tile([128, L], I32)
            nc.gpsimd.iota(ioi, pattern=[[0, L]], base=0,
                           channel_multiplier=1)
            ones_col = cpool.tile([128, 1], F32)
            nc.vector.memset(ones_col, 1.0)
            ones16c = cpool.tile([128, 2], MD)
            nc.vector.memset(ones16c, 1.0)
            onesrow = cpool.tile([1, 128], MD)
            nc.vector.memset(onesrow, 1.0)
            iof = cpool.tile([128, L], F32)
            nc.vector.tensor_copy(iof, ioi)

            # ---- input DMAs (phase-1-critical first on each queue) ----
            g_in = []
            rows_in = []
            for g in range(G):
                arow = rpool.tile([1, N], MD, tag="gin1k", bufs=4,
                                  name="arow")
                nc.sync.dma_start(out=arow, in_=d_atoms[g : g + 1, :])
                prow = rpool.tile([1, L], MD, tag="gin2k", bufs=4,
                                  name="prow")
                nc.sync.dma_start(out=prow, in_=d_amino[g : g + 1, :])
                rows_in.append((arow, prow))

            # ---- weights: 3 blob DMAs + cb ----
            blob16 = cpool.tile([128, W16N], MD, tag="blob16", name="blob16")
            nc.sync.dma_start(out=blob16[:, :708], in_=d_blob16[:, :708])
            nc.scalar.dma_start(out=blob16[:, 708:], in_=d_blob16[:, 708:])
            blobf = cpool.tile([128, WFN], F32, tag="blobf", name="blobf")
            nc.sync.dma_start(out=blobf, in_=d_blobf[:, :])
            blob8 = cpool.tile([128, 128 + LC * NPR * 2 * PD], F8E4,
                               tag="blob8", name="blob8")
            nc.scalar.dma_start(out=blob8, in_=d_blob8[:, :])
            cb = cpool.tile([128, LC], F32)
            nc.gpsimd.dma_start(
                out=cb,
                in_=bass.AP(tensor=d_cb, offset=0,
                            ap=[[0, 128], [1, LC], [0, 1]]),
            )
            for g in range(G):
                ladj8 = apool.tile([128, NT, N], F8, tag=f"ladj{g}",
                                   name="ladj8")
                if g == 0:
                    nc.gpsimd.dma_start(out=ladj8, in_=d_ladj8[g])
                else:
                    nc.scalar.dma_start(out=ladj8, in_=d_ladj8[g])
                amrow = rpool.tile([1, N], F32, tag="gin2kf", bufs=4,
                                   name="amrow")
                nc.gpsimd.dma_start(out=amrow, in_=d_amask[g : g + 1, :])
                pmrow = rpool.tile([1, L], F32, tag="gin4kf", bufs=4,
                                   name="pmrow")
                nc.gpsimd.dma_start(out=pmrow, in_=d_pmask[g : g + 1, :])
                g_in.append((rows_in[g][0], rows_in[g][1], ladj8, amrow,
                             pmrow))

            def w16(nm, n, rows=128):
                return blob16[0:rows, W16O[nm] : W16O[nm] + n]

            Eat = w16("Eat", CD)
            Eam = w16("Eam", PD, NAM)
            Wa2_98 = w16("Wa2_98", 98)
            Wa1_98 = w16("Wa1_98", 98)
            I16 = w16("I16", 128)
            E2 = w16("E2", 128, 33)
            Wc = w16("Wc", LAT)
            Wa = w16("Wa", LAT)
            Wg_flat = w16("Wg", H * GD)
            b98w = blobf[0:98, WFO["b98w"] : WFO["b98w"] + 1]
            b98r = blobf[0:98, WFO["b98r"] : WFO["b98r"] + 1]
            bc = blobf[0:LAT, WFO["bc"] : WFO["bc"] + 1]
            ba = blobf[0:LAT, WFO["ba"] : WFO["ba"] + 1]
            pw = blobf[0:LAT, WFO["pw"] : WFO["pw"] + 2]
            pb = blobf[0:1, WFO["pb"] : WFO["pb"] + 1]

            def MiT_v(lyr, i):
                off = W16O["MiT"] + (lyr * KW + i) * PD
                return blob16[:, off : off + PD]

            def Wgo_v(c):
                off = W16O["Wgo"] + c * CD
                return blob16[:, off : off + CD]

            def Wgoa4_v(c, s):
                off = W16O["Wgoa4"] + c * 4 + s
                return blob16[:, off : off + 2]

            I8 = blob8[:, 0:128]

            def MiT8_v(lyr, pr):
                off = 128 + (lyr * NPR + pr) * 2 * PD
                return bass.AP(tensor=blob8.tensor,
                               offset=blob8.offset + off,
                               ap=[blob8.ap[0], [PD, 2], [1, PD]])

            def leaky_act(out, in_, alpha, bias=None, accum_out=None):
                if use_prelu:
                    kw = {}
                    if bias is not None:
                        kw["bias"] = bias
                    if accum_out is not None:
                        kw["accum_out"] = accum_out
                    nc.scalar.activation(out=out, in_=in_, func=AF.Prelu,
                                         alpha=alpha, **kw)
                    return
                src = in_
                if bias is not None:
                    t = wpool.tile(list(out.shape), F32, tag="lk_t", bufs=4,
                                   name="lkb")
                    nc.scalar.activation(out=t, in_=in_, func=AF.Identity,
                                         bias=bias)
                    src = t
                nc.vector.scalar_tensor_tensor(
                    out=out, in0=src, scalar=alpha, in1=src,
                    op0=AT.mult, op1=AT.max)
                if accum_out is not None:
                    scr = wpool.tile(list(out.shape), F32, tag="lk_t2",
                                     bufs=4, name="lks")
                    nc.vector.tensor_scalar(out=scr, in0=out, scalar1=1.0,
                                            scalar2=0.0, op0=AT.mult,
                                            op1=AT.add, accum_out=accum_out)

            def dump_t(nm, src_ap, shape):
                if not dump:
                    return
                t = wpool.tile(shape, F32, tag="dumpbuf", bufs=2,
                               name="dump" + nm)
                nc.vector.tensor_copy(t, src_ap)
                dst = d_dbg[nm]
                sl = tuple([slice(None)] * len(shape))
                nc.sync.dma_start(out=dst[sl], in_=t)

            st = [dict() for _ in range(G)]

            # ================== phase 1: embeddings ==================
            for g in range(G):
                arow, prow, ladj8, amrow, pmrow = g_in[g]
                # atom one-hot -> avT [CD, N] (broadcast via PE)
                ab_ps = pssq.tile([128, N], F32, tag="sq", name="ab_ps")
                nc.tensor.matmul(ab_ps, onesrow, arow, start=True, stop=True)
                ohA = wpool.tile([128, N], MD, tag="t1k", bufs=6, name="ohA")
                nc.vector.tensor_tensor(out=ohA, in0=ab_ps, in1=iof[:, :N],
                                        op=AT.is_equal)
                avT_ps = pssq.tile([128, N], F32, tag="sq", name="avT_ps")
                nc.tensor.matmul(avT_ps, Eat, ohA, start=True, stop=True)
                avT = wpool.tile([128, N], MD, tag=f"avT{g}", bufs=1,
                                 name="avT")
                nc.scalar.copy(avT, avT_ps)
                st[g]["avT"] = avT

                # amino one-hot -> padded pv
                # dual-plane fp8 pv: plane1[k] = plane0[k+1] so DoubleRow
                # i-pairs read k-tiles at a 16B-aligned plane stride
                if CONV_FP8:
                    pv = apool.tile([PD, 2, PV8W], F8E4, tag=f"pv{g}_0",
                                    bufs=1, name="pv")
                    nc.vector.memset(pv[:, 0, :PADL], 0.0)
                    nc.vector.memset(pv[:, 0, PADL + L :], 0.0)
                    nc.vector.memset(pv[:, 1, : PADL - 1], 0.0)
                    nc.vector.memset(pv[:, 1, PADL - 1 + L :], 0.0)
                else:
                    pv = apool.tile([PD, PVW], MD, tag=f"pv{g}_0", bufs=1,
                                    name="pv")
                    nc.vector.memset(pv[:, :PADL], 0.0)
                    nc.vector.memset(pv[:, PADL + L :], 0.0)
                for c in range(2):
                    pb_ps = pssq.tile([128, N], F32, tag="sq", name="pb_ps")
                    nc.tensor.matmul(pb_ps, onesrow,
                                     prow[:, c * 512 : (c + 1) * 512],
                                     start=True, stop=True)
                    ohP = wpool.tile([NAM, N], MD, tag="t1k", bufs=6,
                                     name="ohP")
                    nc.vector.tensor_tensor(
                        out=ohP, in0=pb_ps[:NAM, :],
                        in1=iof[:NAM, c * 512 : (c + 1) * 512],
                        op=AT.is_equal)
                    pvT_ps = pscv.tile([PD, 512], F32, tag="cv",
                                       name="pvT_ps")
                    nc.tensor.matmul(pvT_ps, Eam, ohP, start=True, stop=True)
                    if CONV_FP8:
                        nc.scalar.copy(
                            pv[:, 0, PADL + c * 512 : PADL + (c + 1) * 512],
                            pvT_ps)
                        nc.vector.tensor_copy(
                            pv[:, 1, PADL - 1 + c * 512 :
                               PADL - 1 + (c + 1) * 512], pvT_ps)
                    else:
                        nc.scalar.copy(
                            pv[:, PADL + c * 512 : PADL + (c + 1) * 512],
                            pvT_ps)
                st[g]["pv"] = pv

            # ================== phase 2: per-graph prep ==================
            for g in range(G):
                avT = st[g]["avT"]
                # all heads' Wh -> whsb_all[p, t, h, 0:64]; col 64 = 1.0
                whsb_all = wpool.tile([128, NT, H, GD + 2], MD,
                                      tag=f"whsb{g}", bufs=1, name="whsb_all")
                nc.vector.memset(whsb_all[:, :, :, GD : GD + 1], 1.0)
                for half in range(2):
                    wh_all = pssq.tile([128, 2, H * GD], F32, tag="sq",
                                       name="wh_all")
                    for t2 in range(2):
                        t = half * 2 + t2
                        nc.tensor.matmul(
                            wh_all[:, t2, :],
                            avT[:, t * 128 : (t + 1) * 128],
                            Wg_flat, start=True, stop=True)
                    nc.scalar.copy(
                        out=bass.AP(
                            tensor=whsb_all.tensor,
                            offset=whsb_all.offset
                            + half * 2 * H * (GD + 2),
                            ap=[whsb_all.ap[0], [H * (GD + 2), 2],
                                [GD + 2, H], [1, GD]]),
                        in_=wh_all)
                st[g]["whsb"] = whsb_all

                # z operands: zws[32h]=dst_h, [32h+1]=1; zrs[32h]=1,
                # [32h+1]=src_h  (heads at legal partition starts)
                zw_t = pssq.tile([128, N], F32, tag="sq", name="zw_ps")
                zw_ps = zw_t[0:98, :]
                nc.tensor.matmul(zw_ps, Wa2_98, avT, start=True, stop=True)
                zws = wpool.tile([98, N], MD, tag=f"zws{g}", bufs=1,
                                 name="zws")
                nc.scalar.activation(out=zws, in_=zw_ps, func=AF.Identity,
                                     bias=b98w)
                zr_t = pssq.tile([128, N], F32, tag="sq", name="zr_ps")
                zr_ps = zr_t[0:98, :]
                nc.tensor.matmul(zr_ps, Wa1_98, avT, start=True, stop=True)
                zrs = wpool.tile([98, N], MD, tag=f"zrs{g}", bufs=1,
                                 name="zrs")
                nc.scalar.activation(out=zrs, in_=zr_ps, func=AF.Identity,
                                     bias=b98r)
                st[g]["zws"] = zws
                st[g]["zrs"] = zrs

            # ================== conv machinery ==================
            conv_steps = []

            def make_conv_layer(lyr):
                pvo_l, cv = [], {}
                last = lyr == LC - 1
                for g in range(G):
                    if last:
                        pvo = apool.tile([PD, PVW], MD, tag=f"pvf{g}",
                                         bufs=1, name="pvo")
                    elif CONV_FP8:
                        pvo = apool.tile([PD, 2, PV8W], F8E4,
                                         tag=f"pv{g}_{1 - lyr % 2}", bufs=1,
                                         name="pvo")
                    else:
                        pvo = apool.tile([PD, PVW], MD,
                                         tag=f"pv{g}_{1 - lyr % 2}", bufs=1,
                                         name="pvo")
                    pvo_l.append(pvo)
                steps = []

                def mk_group(g, c):
                    def run():
                        if c == 0:
                            if last or not CONV_FP8:
                                nc.vector.memset(pvo_l[g][:, :PADL], 0.0)
                                nc.vector.memset(pvo_l[g][:, PADL + L :],
                                                 0.0)
                            else:
                                nc.vector.memset(pvo_l[g][:, 0, :PADL], 0.0)
                                nc.vector.memset(
                                    pvo_l[g][:, 0, PADL + L :], 0.0)
                                nc.vector.memset(
                                    pvo_l[g][:, 1, : PADL - 1], 0.0)
                                nc.vector.memset(
                                    pvo_l[g][:, 1, PADL - 1 + L :], 0.0)
                        cv[g] = pscv.tile([PD, 512], F32, tag="cv",
                                          name=f"cv{g}")
                        pv = st[g]["pv"]
                        if CONV_FP8:
                            for pr in range(NPR):
                                nc.tensor.matmul(
                                    cv[g], MiT8_v(lyr, pr),
                                    bass.AP(tensor=pv.tensor,
                                            offset=pv.offset + c * 512
                                            + 2 * pr,
                                            ap=[pv.ap[0], [PV8W, 2],
                                                [1, 512]]),
                                    start=(pr == 0), stop=(pr == NPR - 1),
                                    perf_mode=DR)
                        else:
                            for i in range(KW):
                                nc.tensor.matmul(
                                    cv[g], MiT_v(lyr, i),
                                    pv[:, c * 512 + i : c * 512 + i + 512],
                                    start=(i == 0), stop=(i == KW - 1))
                    return run

                def mk_relu(g, c):
                    def run():
                        if last or not CONV_FP8:
                            nc.vector.tensor_scalar(
                                out=pvo_l[g][:, PADL + c * 512 :
                                             PADL + (c + 1) * 512],
                                in0=cv[g], scalar1=cb[:, lyr : lyr + 1],
                                scalar2=0.0, op0=AT.add, op1=AT.max)
                        else:
                            nc.vector.tensor_scalar(
                                out=pvo_l[g][:, 0, PADL + c * 512 :
                                             PADL + (c + 1) * 512],
                                in0=cv[g], scalar1=cb[:, lyr : lyr + 1],
                                scalar2=0.0, op0=AT.add, op1=AT.max)
                            nc.vector.tensor_scalar(
                                out=pvo_l[g][:, 1, PADL - 1 + c * 512 :
                                             PADL - 1 + (c + 1) * 512],
                                in0=cv[g], scalar1=cb[:, lyr : lyr + 1],
                                scalar2=0.0, op0=AT.add, op1=AT.max)
                    return run

                for c in range(2):
                    for g in range(G):
                        steps.append(mk_group(g, c))
                        steps.append(mk_relu(g, c))

                def finish():
                    for g in range(G):
                        st[g]["pv"] = pvo_l[g]
                    if dump and lyr == LC - 1:
                        dump_t("pv3", pvo_l[0], [128, PVW])
                steps.append(finish)
                return steps

            for lyr in range(LC):
                conv_steps.extend(make_conv_layer(lyr))

            def pop_conv(k):
                for _ in range(k):
                    if conv_steps:
                        conv_steps.pop(0)()

            # ============ attention z/U (per head or out-layer) ============
            map_idx = [0]

            def att_zU(g, zw2, zr2, dump_zu=False, tp=None):
                """z on PE -> leaky -> exp. zw2/zr2: [2, N] slices
                ([dst;1], [1;src]). Returns U [128, NT, N] fp16."""
                ladj8 = g_in[g][2]
                ee = apool.tile([128, NT, N], MD, tag="ee", bufs=2, name="ee")
                map_idx[0] += 1
                for th in range(2):
                    zps = psz.tile([128, 2, N], F32, tag="z", name="zps")
                    for t2 in range(2):
                        t = th * 2 + t2
                        zc = zps[:, t2, :]
                        nc.tensor.matmul(zc, I8, ladj8[:, t, :],
                                         start=True, stop=False)
                        nc.tensor.matmul(
                            zc, zw2[:, t * 128 : (t + 1) * 128], zr2,
                            start=False, stop=True, tile_position=tp)
                    sl = slice(th * 2, th * 2 + 2)
                    if use_prelu:
                        nc.scalar.activation(out=ee[:, sl, :], in_=zps,
                                             func=AF.Prelu, alpha=ALPHA)
                    else:
                        eesc = wpool.tile([128, 2, N], F32, tag="eesc",
                                          bufs=2, name="eesc")
                        nc.vector.tensor_copy(eesc, zps)
                        nc.vector.scalar_tensor_tensor(
                            out=ee[:, sl, :], in0=eesc, scalar=ALPHA,
                            in1=eesc, op0=AT.mult, op1=AT.max)
                if dump_zu:
                    dump_t("zm0", ee, [128, NT, N])
                U = apool.tile([128, NT, N], MD, tag="U", bufs=2, name="U")
                nc.scalar.activation(out=U, in_=ee, func=AF.Exp)
                if dump_zu:
                    dump_t("U0", U, [128, NT, N])
                return U

            def elu_norm_pair(hp0, hp1, dst):
                """Normalize two heads' hp [65, N] (row 64 = rowsum) and
                elu into packed dst [128, N]."""
                zrw2 = rpool.tile([33, N], F32, tag="zrw2", bufs=3,
                                  name="zrw2")
                nc.gpsimd.memset(zrw2, 1.0)
                nc.vector.tensor_copy(zrw2[0:1, :], hp0[64:65, :])
                nc.vector.tensor_copy(zrw2[32:33, :], hp1[64:65, :])
                rinv2 = rpool.tile([33, N], F32, tag="zrw2", bufs=3,
                                   name="rinv2")
                scr = rpool.tile([33, N], F32, tag="zrw2", bufs=3,
                                 name="rscr")
                nc.vector.reciprocal_approx_accurate(out=rinv2, in_=zrw2,
                                                     scratch=scr)
                rinv16 = rpool.tile([33, N], MD, tag="ri16", bufs=2,
                                    name="rinv16")
                nc.vector.tensor_scalar(out=rinv16, in0=rinv2, scalar1=1.0,
                                        scalar2=None, op0=AT.mult)
                rb_ps = pssq.tile([128, N], F32, tag="sq", name="rb_ps")
                nc.tensor.matmul(rb_ps, E2, rinv16, start=True, stop=True)
                rb = wpool.tile([128, N], MD, tag="rb", bufs=2, name="rb")
                nc.vector.tensor_copy(rb, rb_ps)
                hpn = wpool.tile([128, N], MD, tag="elu", bufs=4, name="hpn")
                nc.vector.scalar_tensor_tensor(
                    out=hpn[0:64, :], in0=hp0[:64, :], scalar=1.0,
                    in1=rb[0:64, :], op0=AT.mult, op1=AT.mult)
                nc.vector.scalar_tensor_tensor(
                    out=hpn[64:128, :], in0=hp1[:64, :], scalar=1.0,
                    in1=rb[64:128, :], op0=AT.mult, op1=AT.mult)
                xm = wpool.tile([128, N], MD, tag="elu", bufs=4, name="xm")
                nc.vector.tensor_scalar(out=xm, in0=hpn, scalar1=0.0,
                                        scalar2=None, op0=AT.min)
                em = wpool.tile([128, N], MD, tag="elu", bufs=4, name="em")
                nc.scalar.activation(out=em, in_=xm, func=AF.Exp)
                nc.vector.scalar_tensor_tensor(
                    out=dst, in0=em, scalar=-1.0, in1=hpn,
                    op0=AT.add, op1=AT.max)

            # ================== phase 3: head maps + conv ==================
            for g in range(G):
                m01 = wpool.tile([128, N], MD, tag=f"m01{g}", bufs=1,
                                 name="m01")
                m23 = wpool.tile([128, N], MD, tag=f"m23{g}", bufs=1,
                                 name="m23")
                st[g]["multi"] = (m01, m23)
                st[g]["hp"] = {}

            # software-pipelined attention: emit z/U of map k+1 before the
            # hp matmuls of map k so the PE never stalls on the exp chain
            pend = None          # (g, h, U) waiting for its hp emission
            pair_hps = {}

            def emit_hp(g, h, U):
                whsb_all = st[g]["whsb"]
                hp_t = pshp.tile([128, N], F32, tag="hp", name="hp_ps")
                hp = hp_t[0:65, :]
                for t in range(NT):
                    nc.tensor.matmul(hp, whsb_all[:, t, h, : GD + 1],
                                     U[:, t, :],
                                     start=(t == 0), stop=(t == NT - 1))
                pair_hps.setdefault(g, []).append(hp)
                if len(pair_hps[g]) == 2:
                    hps = pair_hps.pop(g)
                    elu_norm_pair(hps[0], hps[1], st[g]["multi"][h // 2])

            # warm-up: conv L1 as one dense block before the attention maps
            pop_conv(8)
            for hp_i in range(H // 2):
                for g in range(G):
                    zws, zrs = st[g]["zws"], st[g]["zrs"]
                    for h2 in range(2):
                        h = hp_i * 2 + h2
                        U = att_zU(g, zws[32 * h : 32 * h + 2, :],
                                   zrs[32 * h : 32 * h + 2, :],
                                   dump_zu=(dump and g == 0 and h == 0),
                                   tp=(32 * h, 0))
                        if pend is not None:
                            emit_hp(*pend)
                        pend = (g, h, U)
                        pop_conv(1)
            if pend is not None:
                emit_hp(*pend)
                pend = None

            if dump:
                dump_t("m01", st[0]["multi"][0], [128, N])
                dump_t("m23", st[0]["multi"][1], [128, N])

            # ================== phase 4: GAT output layer ==================
            for g in range(G):
                m01, m23 = st[g]["multi"]
                multi = [m01, m23]
                wh2_ps = pssq.tile([128, NT, CD], F32, tag="sq",
                                   name="wh2_ps")
                for t in range(NT):
                    for c in range(2):
                        nc.tensor.matmul(
                            wh2_ps[:, t, :],
                            multi[c][:, t * 128 : (t + 1) * 128],
                            Wgo_v(c), start=(c == 0), stop=(c == 1))
                wh2sb = wpool.tile([128, NT, CD], MD, tag=f"wh2{g}", bufs=1,
                                   name="wh2sb")
                nc.vector.tensor_copy(wh2sb, wh2_ps)
                st[g]["wh2sb"] = wh2sb

                zw2_t = pssq.tile([128, N], F32, tag="sq", name="zw2_ps")
                zw2_ps = zw2_t[0:2, :]
                for c in range(2):
                    nc.tensor.matmul(zw2_ps, Wgoa4_v(c, 0), multi[c],
                                     start=(c == 0), stop=(c == 1))
                zw2 = rpool.tile([2, N], MD, tag="zw2", bufs=4, name="zw2")
                b2w = rpool.tile([2, 1], F32, tag="b2", bufs=4, name="b2w")
                nc.vector.memset(b2w, 1.0)
                nc.vector.memset(b2w[0:1, :], 0.0)
                nc.scalar.activation(out=zw2, in_=zw2_ps, func=AF.Identity,
                                     bias=b2w)
                zr2_t = pssq.tile([128, N], F32, tag="sq", name="zr2_ps")
                zr2_ps = zr2_t[0:2, :]
                for c in range(2):
                    nc.tensor.matmul(zr2_ps, Wgoa4_v(c, 2), multi[c],
                                     start=(c == 0), stop=(c == 1))
                zr2 = rpool.tile([2, N], MD, tag="zw2", bufs=4, name="zr2")
                b2r = rpool.tile([2, 1], F32, tag="b2", bufs=4, name="b2r")
                nc.vector.memset(b2r, 0.0)
                nc.vector.memset(b2r[0:1, :], 1.0)
                nc.scalar.activation(out=zr2, in_=zr2_ps, func=AF.Identity,
                                     bias=b2r)
                st[g]["zw2"] = zw2
                st[g]["zr2"] = zr2
                pop_conv(3)

            oU, ohp, ors, orinv, orb = {}, {}, {}, {}, {}
            for g in range(G):
                oU[g] = att_zU(g, st[g]["zw2"], st[g]["zr2"])
                pop_conv(1)
            for g in range(G):
                wh2sb = st[g]["wh2sb"]
                hp2 = pshp.tile([128, N], F32, tag="hp", name="hp2_ps")
                for t in range(NT):
                    nc.tensor.matmul(hp2, wh2sb[:, t, :], oU[g][:, t, :],
                                     start=(t == 0), stop=(t == NT - 1))
                rs_t = psz.tile([128, 2, N], F32, tag="z", name="rs_ps")
                rs_ps = rs_t[:, 0, :]
                for t in range(NT):
                    nc.tensor.matmul(rs_ps[0:1, :], ones16c[:, 0:1],
                                     oU[g][:, t, :],
                                     start=(t == 0), stop=(t == NT - 1))
                ohp[g] = hp2
                ors[g] = rs_ps
                pop_conv(1)
            for g in range(G):
                zrw = rpool.tile([1, N], F32, tag="zrwo", bufs=6, name="zrw")
                nc.scalar.copy(zrw, ors[g][0:1, :])
                rinv = rpool.tile([1, N], F32, tag="zrwo", bufs=6,
                                  name="rinv")
                scr = rpool.tile([1, N], F32, tag="zrwo", bufs=6,
                                 name="rscr")
                nc.vector.reciprocal_approx_accurate(out=rinv, in_=zrw,
                                                     scratch=scr)
                if dump and g == 0:
                    dump_t("rinv0", rinv, [1, N])
                rinv16 = rpool.tile([1, N], MD, tag="ri16o", bufs=2,
                                    name="rinv16o")
                nc.vector.tensor_scalar(out=rinv16, in0=rinv, scalar1=1.0,
                                        scalar2=None, op0=AT.mult)
                orinv[g] = rinv16
            for g in range(G):
                rb_ps = pssq.tile([128, N], F32, tag="sq", name="rb2_ps")
                nc.tensor.matmul(rb_ps, onesrow, orinv[g], start=True,
                                 stop=True)
                rb = wpool.tile([128, N], MD, tag="rb", bufs=2, name="rbo")
                nc.vector.tensor_copy(rb, rb_ps)
                orb[g] = rb
            for g in range(G):
                xT = wpool.tile([CD, N], MD, tag=f"xT{g}", bufs=1, name="xT")
                hpn = wpool.tile([128, N], MD, tag="elu", bufs=4,
                                 name="hpno")
                nc.vector.scalar_tensor_tensor(
                    out=hpn, in0=ohp[g], scalar=1.0, in1=orb[g],
                    op0=AT.mult, op1=AT.mult)
                xm = wpool.tile([128, N], MD, tag="elu", bufs=4, name="xmo")
                nc.vector.tensor_scalar(out=xm, in0=hpn, scalar1=0.0,
                                        scalar2=None, op0=AT.min)
                em = wpool.tile([128, N], MD, tag="elu", bufs=4, name="emo")
                nc.scalar.activation(out=em, in_=xm, func=AF.Exp)
                nc.vector.scalar_tensor_tensor(
                    out=xT, in0=em, scalar=-1.0, in1=hpn,
                    op0=AT.add, op1=AT.max)
                st[g]["xT"] = xT
                if dump and g == 0:
                    dump_t("xT", xT, [128, N])
                pop_conv(2)

            # ============ phase 5: comp head + conv (graph-stepped) ========
            oav, oavec, oav2 = {}, {}, {}
            for g in range(G):
                av_t = pshp.tile([128, N], F32, tag="hp", name="av_ps")
                oav[g] = av_t[:LAT, :]
                nc.tensor.matmul(oav[g], Wc, st[g]["xT"], start=True,
                                 stop=True)
                pop_conv(1)
            for g in range(G):
                avec = wpool.tile([LAT, N], MD, tag="avec", bufs=2,
                                  name="avec")
                leaky_act(avec, oav[g], ALPHA, bias=bc)
                oavec[g] = avec
            for g in range(G):
                av2_t = pshp.tile([128, N], F32, tag="hp", name="av2_ps")
                oav2[g] = av2_t[:LAT, :]
                nc.tensor.matmul(oav2[g], Wa, oavec[g], start=True,
                                 stop=True)
                pop_conv(1)
            for g in range(G):
                a_v = wpool.tile([LAT, N], MD, tag="avec", bufs=2,
                                 name="a_v")
                comp_acc = rpool.tile([LAT, 1], F32, tag="c1", bufs=8,
                                      name="comp_acc")
                leaky_act(a_v, oav2[g], ALPHA, bias=ba, accum_out=comp_acc)
                st[g]["comp_acc"] = comp_acc
            for g in range(G):
                comp_acc = st[g]["comp_acc"]
                amrow = g_in[g][3]
                amscr = rpool.tile([1, N], F32, tag="r2k", bufs=6,
                                   name="amscr")
                amsum = rpool.tile([1, 1], F32, tag="c2", bufs=12,
                                   name="amsum")
                nc.vector.tensor_scalar(out=amscr, in0=amrow, scalar1=1.0,
                                        scalar2=0.0, op0=AT.mult, op1=AT.add,
                                        accum_out=amsum)
                amsb = rpool.tile([128, 1], F32, tag="c2", bufs=12,
                                  name="amsb")
                nc.gpsimd.partition_broadcast(amsb, amsum)
                amr = rpool.tile([128, 1], F32, tag="c2", bufs=12,
                                 name="amr")
                nc.vector.reciprocal(amr, amsb)
                cp = rpool.tile([128, 2], F32, tag="cp", bufs=4, name="cp")
                nc.vector.tensor_scalar(out=cp[:, 0:1], in0=comp_acc,
                                        scalar1=amr, scalar2=None,
                                        op0=AT.mult)
                st[g]["cp"] = cp
                if dump and g == 0:
                    dump_t("comp", comp_acc, [LAT, 1])
                pop_conv(3)

            pop_conv(len(conv_steps))

            # ========= phase 6: prot head + prediction (graph-stepped) =====
            opacc = {}
            for c in range(2):
                for g in range(G):
                    pv = st[g]["pv"]
                    pvt = psz.tile([128, 2, N], F32, tag="z", name="pv_ps")
                    pv_ps = pvt[:LAT, 0, :]
                    nc.tensor.matmul(pv_ps, Wa,
                                     pv[:, PADL + c * 512 :
                                        PADL + (c + 1) * 512],
                                     start=True, stop=True)
                    p_v = wpool.tile([LAT, 512], MD, tag="p_v", bufs=4,
                                     name="p_v")
                    pacc = rpool.tile([LAT, 1], F32, tag="c1", bufs=8,
                                      name="pacc")
                    leaky_act(p_v, pv_ps, ALPHA, bias=ba, accum_out=pacc)
                    opacc.setdefault(g, []).append(pacc)
            for g in range(G):
                cp = st[g]["cp"]
                prot_acc = rpool.tile([LAT, 1], F32, tag="c1", bufs=8,
                                      name="prot_acc")
                nc.vector.tensor_tensor(out=prot_acc, in0=opacc[g][0],
                                        in1=opacc[g][1], op=AT.add)
                if dump and g == 0:
                    dump_t("prot", prot_acc, [LAT, 1])

                pmrow = g_in[g][4]
                pmscr = rpool.tile([1, L], F32, tag="r4k", bufs=2,
                                   name="pmscr")
                pmsum = rpool.tile([1, 1], F32, tag="c2", bufs=12,
                                   name="pmsum")
                nc.vector.tensor_scalar(out=pmscr, in0=pmrow, scalar1=1.0,
                                        scalar2=0.0, op0=AT.mult, op1=AT.add,
                                        accum_out=pmsum)
                pmsb = rpool.tile([128, 1], F32, tag="c2", bufs=12,
                                  name="pmsb")
                nc.gpsimd.partition_broadcast(pmsb, pmsum)
                pmr = rpool.tile([128, 1], F32, tag="c2", bufs=12, name="pmr")
                nc.vector.reciprocal(pmr, pmsb)
                nc.vector.tensor_scalar(out=cp[:, 1:2], in0=prot_acc,
                                        scalar1=pmr, scalar2=None,
                                        op0=AT.mult)

                lr2 = rpool.tile([128, 2], F32, tag="cp", bufs=4, name="lr2")
                leaky_act(lr2, cp, ALPHA * ALPHA)
                dscr = rpool.tile([128, 2], F32, tag="cp", bufs=4,
                                  name="dscr")
                dacc = rpool.tile([128, 1], F32, tag="c1", bufs=8,
                                  name="dacc")
                nc.vector.scalar_tensor_tensor(
                    out=dscr, in0=lr2, scalar=1.0, in1=pw,
                    op0=AT.mult, op1=AT.mult, accum_out=dacc)
                fin_ps = pssq.tile([128, N], F32, tag="sq", name="fin_ps")
                nc.tensor.matmul(fin_ps[0:1, 0:1], dacc, ones_col,
                                 start=True, stop=True)
                res = rpool.tile([1, 1], F32, tag="c2", bufs=12, name="res")
                nc.scalar.activation(out=res, in_=fin_ps[0:1, 0:1],
                                     func=AF.Identity, bias=pb)
                nc.sync.dma_start(out=d_out[g : g + 1, :], in_=res)

    return nc


def preprocess(inputs, mm_bf16=False):
    """Host-side prep: shard over cores, transpose/reshape weights."""
    import ml_dtypes
    md = np.float16
    f8 = ml_dtypes.float8_e5m2
    atoms = np.asarray(inputs["atoms"]).astype(np.float32)
    atoms_mask = np.asarray(inputs["atoms_mask"]).astype(np.float32)
    adjacency = np.asarray(inputs["adjacency"])
    amino = np.asarray(inputs["amino"]).astype(np.float32)
    amino_mask = np.asarray(inputs["amino_mask"]).astype(np.float32)
    E_atom = np.asarray(inputs["E_atom"]).astype(np.float32)
    E_amino = np.asarray(inputs["E_amino"]).astype(np.float32)
    W_gat = np.asarray(inputs["W_gat"]).astype(np.float32)
    a_gat = np.asarray(inputs["a_gat"]).astype(np.float32)
    W_go = np.asarray(inputs["W_go"]).astype(np.float32)
    a_go = np.asarray(inputs["a_go"]).astype(np.float32)
    W_comp_w = np.asarray(inputs["W_comp_w"]).astype(np.float32)
    W_comp_b = np.asarray(inputs["W_comp_b"]).astype(np.float32)
    conv_w = np.asarray(inputs["conv_w"]).astype(np.float32)
    conv_b = np.asarray(inputs["conv_b"]).astype(np.float32)
    W_att_w = np.asarray(inputs["W_att_w"]).astype(np.float32)
    W_att_b = np.asarray(inputs["W_att_b"]).astype(np.float32)
    pred_w = np.asarray(inputs["pred_w"]).astype(np.float32)
    pred_b = np.asarray(inputs["pred_b"]).astype(np.float32)

    ladjT = np.where(adjacency.transpose(0, 2, 1) > 0, np.float32(0.0),
                     np.float32(MASKNEG)).astype(np.float32)
    ladjT_r = np.ascontiguousarray(
        ladjT.reshape(B, NT, 128, N).transpose(0, 2, 1, 3))

    E_atom_pad = np.zeros((128, CD), np.float32)
    E_atom_pad[:NA] = E_atom

    MiT = np.zeros((LC, KW, PD, PD), np.float32)
    din = np.arange(PD)[:, None]
    dout = np.arange(PD)[None, :]
    v = din - dout + (KW // 2)
    valid = (v >= 0) & (v < KW)
    vc = np.clip(v, 0, KW - 1)
    for lyr in range(LC):
        for i in range(KW):
            MiT[lyr, i] = np.where(valid, conv_w[lyr, 0, 0, i, vc], 0.0)
    MiT_r = np.ascontiguousarray(MiT.transpose(2, 0, 1, 3))
    NPRl = (KW + 1) // 2
    MiT8 = np.zeros((LC, NPRl, 2, PD, PD), np.float32)
    for lyr in range(LC):
        for pr in range(NPRl):
            MiT8[lyr, pr, 0] = MiT[lyr, 2 * pr]
            if 2 * pr + 1 < KW:
                MiT8[lyr, pr, 1] = MiT[lyr, 2 * pr + 1]
    MiT8_r = np.ascontiguousarray(MiT8.transpose(3, 0, 1, 2, 4))

    W_gat_r = np.ascontiguousarray(W_gat.transpose(1, 0, 2))
    Wa1h = np.einsum("hpq,hq->ph", W_gat, a_gat[:, :GD])
    Wa2h = np.einsum("hpq,hq->ph", W_gat, a_gat[:, GD:])
    Wa2_98 = np.zeros((CD, 98), np.float32)
    Wa2_98[:, [0, 32, 64, 96]] = Wa2h
    Wa1_98 = np.zeros((CD, 98), np.float32)
    Wa1_98[:, [1, 33, 65, 97]] = Wa1h
    b98w = np.zeros((98, 1), np.float32)
    b98w[[1, 33, 65, 97]] = 1.0
    b98r = np.zeros((98, 1), np.float32)
    b98r[[0, 32, 64, 96]] = 1.0

    W_go_r = np.ascontiguousarray(
        W_go.reshape(2, 128, CD).transpose(1, 0, 2))
    Wgo_a2 = (W_go @ a_go[CD:]).reshape(2, 128).T     # (128, 2)
    Wgo_a1 = (W_go @ a_go[:CD]).reshape(2, 128).T
    Wgoa4 = np.zeros((128, 2, 4), np.float32)
    Wgoa4[:, :, 0] = Wgo_a2
    Wgoa4[:, :, 3] = Wgo_a1

    I8dr = np.zeros((128, 2, 128), np.float32)
    I8dr[:, 0, :] = np.eye(128, dtype=np.float32)
    I16 = np.eye(128, dtype=np.float32)
    E2 = np.zeros((33, 128), np.float32)
    E2[0, 0:64] = 1.0
    E2[32, 64:128] = 1.0

    # pack fp16 weight blob (offsets mirror kernel W16O)
    W16N = 1484 + LC * KW * PD
    blob16 = np.zeros((128, W16N), np.float32)

    def put16(off, arr):
        a = np.asarray(arr, np.float32)
        blob16[: a.shape[0], off : off + a.shape[1]] = a

    put16(0, E_atom_pad)
    put16(128, E_amino)
    put16(256, W_gat_r.reshape(CD, H * GD))
    put16(512, Wa2_98)
    put16(610, Wa1_98)
    put16(708, W_go_r.reshape(128, 2 * CD))
    put16(964, Wgoa4.reshape(128, 8))
    put16(972, I16)
    put16(1100, E2)
    put16(1228, W_comp_w.T)
    put16(1356, W_att_w.T)
    put16(1484, MiT_r.reshape(PD, LC * KW * PD))

    blobf = np.zeros((128, 7), np.float32)
    blobf[:98, 0] = b98w[:, 0]
    blobf[:98, 1] = b98r[:, 0]
    blobf[:LAT, 2] = W_comp_b
    blobf[:LAT, 3] = W_att_b
    blobf[:LAT, 4] = pred_w[0, :LAT]
    blobf[:LAT, 5] = pred_w[0, LAT:]
    blobf[0, 6] = pred_b[0]

    blob8 = np.zeros((128, 128 + LC * NPRl * 2 * PD), np.float32)
    blob8[:, :128] = I16
    blob8[:, 128:] = MiT8_r.reshape(PD, LC * NPRl * 2 * PD)

    shared = {
        "blob16": blob16.astype(md),
        "blobf": blobf,
        "blob8": blob8.astype(ml_dtypes.float8_e4m3fn),
        "conv_b": np.ascontiguousarray(conv_b.reshape(LC, 1)),
    }
    in_maps = []
    for c in range(NCORES):
        sl = slice(c * G, (c + 1) * G)
        m = dict(shared)
        m["atoms_f"] = np.ascontiguousarray(atoms[sl]).astype(md)
        m["atoms_mask"] = np.ascontiguousarray(atoms_mask[sl])
        m["ladj8"] = np.ascontiguousarray(ladjT_r[sl]).astype(f8)
        m["amino_f"] = np.ascontiguousarray(amino[sl]).astype(md)
        m["amino_mask"] = np.ascontiguousarray(amino_mask[sl])
        in_maps.append(m)
    return in_maps


_CACHED_NC = None


def kernel(**inputs) -> np.ndarray:
    global _CACHED_NC
    from concourse.bass_utils import run_bass_kernel_spmd

    if _CACHED_NC is None:
        nc = build_core_program()
        nc.finalize()
        _CACHED_NC = nc
    nc = _CACHED_NC
    in_maps = preprocess(inputs)
    res = run_bass_kernel_spmd(nc, in_maps, core_ids=list(range(NCORES)))
    out = np.concatenate([res.results[c]["out"] for c in range(NCORES)], axis=0)
    return out.astype(np.float32)
